# revision 1
# baseline (speedup 1.0000x reference)
"""MultiHeadedAttention Trainium2 Bass kernel.

Full inputs in, full output out. Sharding: 8 cores = 4 batches x 2 head-pairs
(data-parallel over batch, tensor-parallel over the 4 heads). Per core, all
matmuls in bf16 (fp32 PSUM accumulation):
  Q/K projections for its 2 heads      -> [128, 2048] bf16 (chan-major)
  V projection directly transposed     -> vt [m, (h, d+ones)] bf16
  per head: scoresT[m,n] = K^T Q, exp on ACT (scale=1/8; no max-sub needed,
  |s/8| < ~5), x[d+1, n] accumulated over m in PSUM with vt as the stationary
  operand (ones row gives softmax sums), normalize via DMA-broadcast 1/sums,
  out projection with both heads accumulated in PSUM.
Host pre-casts inputs/weights to bf16, sums the two per-batch partials and
adds the output bias in fp32.
"""

import sys

if "/opt/trn_rl_repo" not in sys.path:
    sys.path.insert(0, "/opt/trn_rl_repo")

import numpy as np
import ml_dtypes

BF = ml_dtypes.bfloat16

B, D, N, H = 4, 256, 2048, 4
DIM = D // H  # 64
NW = 4  # 512-wide n windows
MB = 16  # 128-wide m blocks

_CACHE = {}


def _emit(ctx, tc, io):
    import concourse.bass as bass
    import concourse.mybir as mybir

    nc = tc.nc
    f32 = mybir.dt.float32
    bf16 = mybir.dt.bfloat16
    EXP = mybir.ActivationFunctionType.Exp
    LN = mybir.ActivationFunctionType.Ln

    const = ctx.enter_context(tc.tile_pool(name="const", bufs=1))
    xin = ctx.enter_context(tc.tile_pool(name="xin", bufs=4))
    big = ctx.enter_context(tc.tile_pool(name="big", bufs=1))
    xpool = ctx.enter_context(tc.tile_pool(name="xpool", bufs=2))
    pb = ctx.enter_context(tc.tile_pool(name="probs", bufs=3))
    work = ctx.enter_context(tc.tile_pool(name="work", bufs=2))
    outp = ctx.enter_context(tc.tile_pool(name="outp", bufs=3))
    psA = ctx.enter_context(tc.tile_pool(name="psA", bufs=2, space="PSUM"))
    psX = ctx.enter_context(tc.tile_pool(name="psX", bufs=4, space="PSUM"))
    dpool = ctx.enter_context(tc.tile_pool(name="dpool", bufs=2, space="DRAM"))

    # ---- constants / weights (all bf16 except the f32 Q/K biases) ----
    wqt_sb = const.tile([128, 2, 128], bf16, tag="wqt")
    nc.sync.dma_start(wqt_sb, io["wqt"].rearrange("(c p) o -> p c o", p=128))
    wkt_sb = const.tile([128, 2, 128], bf16, tag="wkt")
    nc.sync.dma_start(wkt_sb, io["wkt"].rearrange("(c p) o -> p c o", p=128))
    wvt_sb = const.tile([128, 2, 128], bf16, tag="wvt")
    nc.sync.dma_start(wvt_sb, io["wvt"].rearrange("(c p) o -> p c o", p=128))
    wmt0_sb = const.tile([64, 256], bf16, tag="wmt0")
    nc.sync.dma_start(wmt0_sb, io["wmt0"])
    wmt1_sb = const.tile([64, 256], bf16, tag="wmt1")
    nc.sync.dma_start(wmt1_sb, io["wmt1"])
    bq_sb = const.tile([128, 1], f32, tag="bq")
    nc.sync.dma_start(bq_sb, io["bq"])
    bk_sb = const.tile([128, 1], f32, tag="bk")
    nc.sync.dma_start(bk_sb, io["bk"])
    bv_sb = const.tile([1, 128], bf16, tag="bv")
    nc.sync.dma_start(bv_sb, io["bv"])
    onesb = const.tile([1, 128], bf16, tag="onesb")
    nc.gpsimd.memset(onesb, 1.0)
    ones64f = const.tile([1, 64], f32, tag="ones64f")
    nc.gpsimd.memset(ones64f, 1.0)

    # PE warmup: the HAM clock gate only releases (1.2 -> 2.4 GHz) after a
    # fully-busy ~3.4us window. Run a burst of back-to-back matmuls during the
    # input-DMA ramp so the attention phase starts (and stays) warm.
    wu_a = const.tile([128, 128], bf16, tag="wu_a")
    nc.gpsimd.memset(wu_a, 0.0)
    wu_b = const.tile([128, 512], bf16, tag="wu_b")
    nc.gpsimd.memset(wu_b, 0.0)
    wu_ps = psA.tile([128, 1024], f32, tag="ps", name="wu_ps")
    for i in range(48):
        nc.tensor.matmul(wu_ps[:, 0:512], lhsT=wu_a, rhs=wu_b, start=True, stop=True)

    # ---- input loads (chunked 2x per tile for DMA-queue spread, issued from
    # three different HWDGE engines so the rings run in parallel) ----
    xq_t, xk_t, xv_t = [], [], []
    eng = {"xq": nc.sync, "xk": nc.scalar, "xv": nc.sync}
    for w in range(NW):
        for name, lst in (("xq", xq_t), ("xk", xk_t), ("xv", xv_t)):
            t = xin.tile([128, 2, 512], bf16, tag=name, name=f"{name}{w}")
            src = io[name].rearrange("(c p) n -> p c n", p=128)
            for hh in range(2):
                s = slice(w * 512 + hh * 256, w * 512 + (hh + 1) * 256)
                eng[name].dma_start(t[:, :, hh * 256 : (hh + 1) * 256], src[:, :, s])
            lst.append(t)

    # ---- phases 1+2 interleaved: emit K-w0, Q-w0/w1 and the first four
    # V^T blocks first so the attention m-loop can start while the rest of
    # the projections and V^T blocks still stream in.
    q_sb = big.tile([128, 2048], bf16, tag="q")
    k_sb = big.tile([128, 2048], bf16, tag="k")

    def proj_step(xt, wt, bias, dst, w):
        ps = psA.tile([128, 1024], f32, tag="ps", name=f"psproj{w}")
        nc.tensor.matmul(ps[:, 0:512], lhsT=wt[:, 0, :], rhs=xt[w][:, 0, :], start=True, stop=False)
        nc.tensor.matmul(ps[:, 0:512], lhsT=wt[:, 1, :], rhs=xt[w][:, 1, :], start=False, stop=True)
        nc.vector.tensor_scalar_add(dst[:, w * 512 : (w + 1) * 512], ps[:, 0:512], bias)

    vt = big.tile([128, MB, 2, 65], bf16, tag="vt")
    nc.gpsimd.memset(vt[:, :, :, 64:65], 1.0)

    def vt_step(mb):
        w, off = divmod(mb, 4)
        ms = slice(off * 128, (off + 1) * 128)
        ps = psA.tile([128, 1024], f32, tag="ps", name=f"psvt{mb}")
        pvt = ps[:, 0:128]
        nc.tensor.matmul(pvt, lhsT=onesb, rhs=bv_sb, start=True, stop=False)
        nc.tensor.matmul(pvt, lhsT=xv_t[w][:, 0, ms], rhs=wvt_sb[:, 0, :], start=False, stop=False)
        nc.tensor.matmul(pvt, lhsT=xv_t[w][:, 1, ms], rhs=wvt_sb[:, 1, :], start=False, stop=True)
        nc.vector.tensor_copy(vt[:, mb, :, 0:64], pvt.rearrange("m (h d) -> m h d", h=2))

    proj_step(xk_t, wkt_sb, bk_sb, k_sb, 0)
    proj_step(xq_t, wqt_sb, bq_sb, q_sb, 0)
    proj_step(xq_t, wqt_sb, bq_sb, q_sb, 1)
    for mb in range(4):
        vt_step(mb)
    proj_step(xk_t, wkt_sb, bk_sb, k_sb, 1)
    proj_step(xq_t, wqt_sb, bq_sb, q_sb, 2)
    proj_step(xk_t, wkt_sb, bk_sb, k_sb, 2)
    proj_step(xq_t, wqt_sb, bq_sb, q_sb, 3)
    proj_step(xk_t, wkt_sb, bk_sb, k_sb, 3)
    for mb in range(4, MB):
        vt_step(mb)

    # ---- phase 3: attention per head ----
    # PE-order grouping: without explicit deps the scheduler alternates
    # scores and x-accum matmuls, forcing a LDWEIGHTS before every matmul.
    # Enforce [4 scores of mb+1][4 x-accums of mb] alternation instead.
    from concourse.tile_rust import add_dep_helper

    def _raw(inst):
        return getattr(inst, "ins", inst)

    x_sb = []
    sc_groups = []  # scores-matmul groups in emission order across heads/halves
    xa_groups = []
    for h in range(2):
        qh = q_sb[h * 64 : (h + 1) * 64, :]
        kh = k_sb[h * 64 : (h + 1) * 64, :]
        xh = xpool.tile([64, 2048], bf16, tag="x", name=f"x{h}")
        # n split in two 1024 halves: x-accum psum drops to 2 banks, which
        # lets the scores psum triple-buffer (3x2 + 2 = 8 banks) so the PE
        # never stalls on exp.
        for nh in range(2):
            nbase = nh * 1024
            px = [psX.tile([65, 512], f32, tag="px", name=f"px{h}_{nh}_{i}") for i in range(2)]
            for mb in range(MB):
                pt = pb.tile([128, 1024], bf16, tag="pt", name="pt")
                sc = psA.tile([128, 1024], f32, tag="ps", name="pssc")
                scg = []
                for s2 in range(2):
                    n0 = nbase + s2 * 512
                    scg.append(nc.tensor.matmul(
                        sc[:, s2 * 512 : (s2 + 1) * 512],
                        lhsT=kh[:, mb * 128 : (mb + 1) * 128],
                        rhs=qh[:, n0 : n0 + 512],
                        start=True,
                        stop=True,
                    ))
                nc.scalar.activation(pt, sc, EXP, scale=0.125)
                sc_groups.append(scg)
                xag = []
                for j in range(2):
                    xag.append(nc.tensor.matmul(
                        px[j],
                        lhsT=vt[:, mb, h, :],
                        rhs=pt[:, j * 512 : (j + 1) * 512],
                        start=(mb == 0),
                        stop=(mb == MB - 1),
                        skip_group_check=True,
                    ))
                # filler matmul: reuses the vt stationary (no LDWEIGHTS) and
                # writes a dead region of the just-consumed scores bank. Keeps
                # the PE's HAM activity window busy so the 2.4GHz clock gate
                # stays open once the warmup burst releases it.
                nc.tensor.matmul(
                    sc[0:65, 0:512],
                    lhsT=vt[:, mb, h, :],
                    rhs=wu_b,
                    start=True,
                    stop=True,
                    skip_group_check=True,
                )
                xa_groups.append(xag)

            # normalize this half: broadcast sums via DRAM bounce, then
            # multi-lane DVE reciprocal + multiply.
            s_row = work.tile([1, 1024], f32, tag="s_row", name=f"s_row{h}_{nh}")
            for j in range(2):
                nc.scalar.copy(s_row[:, j * 512 : (j + 1) * 512], px[j][64:65, :])
            s_dram = dpool.tile([1, 1024], f32, tag="s_dram", name=f"s_dram{h}_{nh}")
            nc.sync.dma_start(s_dram, s_row)
            s_bc = work.tile([64, 1024], f32, tag="s_bc", name=f"s_bc{h}_{nh}")
            s_src = bass.AP(
                tensor=s_dram.tensor,
                offset=s_dram.offset,
                ap=[[0, 64]] + list(s_dram.ap[1:]),
            )
            nc.sync.dma_start(s_bc, s_src)
            r_bc = work.tile([64, 1024], f32, tag="r_bc", name=f"r_bc{h}_{nh}")
            nc.vector.reciprocal(r_bc, s_bc)
            for j in range(2):
                nc.vector.tensor_mul(
                    xh[:, nbase + j * 512 : nbase + (j + 1) * 512],
                    px[j][0:64, :],
                    r_bc[:, j * 512 : (j + 1) * 512],
                )
        x_sb.append(xh)

    # PE alternation deps: xa[g] after sc[g+1]; sc[g+2] after xa[g]
    G = len(sc_groups)
    for g in range(G):
        if g + 1 < G:
            for m in xa_groups[g]:
                add_dep_helper(_raw(m), _raw(sc_groups[g + 1][-1]), False,
                               "group x-accums after next scores")
        if g + 2 < G:
            for m in sc_groups[g + 2]:
                add_dep_helper(_raw(m), _raw(xa_groups[g][-1]), False,
                               "group scores after prev x-accums")

    if "dbg_q" in io:
        nc.sync.dma_start(io["dbg_q"], q_sb)
        nc.sync.dma_start(io["dbg_k"], k_sb)
        nc.sync.dma_start(io["dbg_vt"], vt)
        nc.sync.dma_start(io["dbg_x0"], x_sb[0])
        nc.sync.dma_start(io["dbg_x1"], x_sb[1])

    # ---- phase 4: out projection, heads accumulated in PSUM ----
    for oc in range(2):
        ocs = slice(oc * 128, (oc + 1) * 128)
        po = [psA.tile([128, 1024], f32, tag="ps", name=f"po{oc}_{g}") for g in range(2)]
        for g in range(2):
            for s2 in range(2):
                w = g * 2 + s2
                nc.tensor.matmul(po[g][:, s2 * 512 : (s2 + 1) * 512], lhsT=wmt0_sb[:, ocs],
                                 rhs=x_sb[0][:, w * 512 : (w + 1) * 512], start=True, stop=False)
        for g in range(2):
            for s2 in range(2):
                w = g * 2 + s2
                nc.tensor.matmul(po[g][:, s2 * 512 : (s2 + 1) * 512], lhsT=wmt1_sb[:, ocs],
                                 rhs=x_sb[1][:, w * 512 : (w + 1) * 512], start=False, stop=True)
        for g in range(2):
            ws = slice(g * 1024, (g + 1) * 1024)
            ot = outp.tile([128, 1024], f32, tag="ot", name="ot")
            nc.vector.tensor_copy(ot, po[g])
            nc.sync.dma_start(io["out"][ocs, ws], ot)


def _build_nc(debug_dumps=False):
    key = ("nc", debug_dumps)
    if key in _CACHE:
        return _CACHE[key]
    from contextlib import ExitStack

    import concourse.mybir as mybir
    import concourse.tile as tile
    from concourse import bacc

    f32 = mybir.dt.float32
    bf16 = mybir.dt.bfloat16
    nc = bacc.Bacc("TRN2", target_bir_lowering=False, debug=False, num_devices=8)
    io = {}
    for name, shape, dt_ in (
        ("xq", [256, 2048], bf16),
        ("xk", [256, 2048], bf16),
        ("xv", [256, 2048], bf16),
        ("wqt", [256, 128], bf16),
        ("wkt", [256, 128], bf16),
        ("wvt", [256, 128], bf16),
        ("bq", [128, 1], f32),
        ("bk", [128, 1], f32),
        ("bv", [1, 128], bf16),
        ("wmt0", [64, 256], bf16),
        ("wmt1", [64, 256], bf16),
    ):
        io[name] = nc.dram_tensor(name, shape, dt_, kind="ExternalInput").ap()
    io["out"] = nc.dram_tensor("out", [256, 2048], f32, kind="ExternalOutput").ap()
    if debug_dumps:
        io["dbg_q"] = nc.dram_tensor("dbg_q", [128, 2048], bf16, kind="ExternalOutput").ap()
        io["dbg_k"] = nc.dram_tensor("dbg_k", [128, 2048], bf16, kind="ExternalOutput").ap()
        io["dbg_vt"] = nc.dram_tensor("dbg_vt", [128, MB, 2, 65], bf16, kind="ExternalOutput").ap()
        io["dbg_x0"] = nc.dram_tensor("dbg_x0", [64, 2048], bf16, kind="ExternalOutput").ap()
        io["dbg_x1"] = nc.dram_tensor("dbg_x1", [64, 2048], bf16, kind="ExternalOutput").ap()

    with tile.TileContext(nc) as tc:
        with ExitStack() as ctx:
            _emit(ctx, tc, io)
    nc.compile()
    _CACHE[key] = nc
    _CACHE[(key, "io")] = io
    return nc


def make_in_maps(query, key, value, wq, bq, wk, bk, wv, bv, wm, bm):
    fb = lambda a: np.ascontiguousarray(np.asarray(a, dtype=np.float32)).astype(BF)
    f = lambda a: np.ascontiguousarray(np.asarray(a), dtype=np.float32)
    query, key, value = f(query), f(key), f(value)
    wq, wk, wv, wm = f(wq), f(wk), f(wv), f(wm)
    bq, bk, bv = f(bq), f(bk), f(bv)
    in_maps = []
    for c in range(8):
        b, pair = divmod(c, 2)
        hs = (2 * pair, 2 * pair + 1)
        idx = np.array([d * H + h for h in hs for d in range(DIM)])
        m = {
            "xq": fb(query[b]),
            "xk": fb(key[b]),
            "xv": fb(value[b]),
            "wqt": fb(wq[idx].T),
            "wkt": fb(wk[idx].T),
            "wvt": fb(wv[idx].T),
            "bq": f(bq[idx].reshape(128, 1)),
            "bk": f(bk[idx].reshape(128, 1)),
            "bv": fb(bv[idx].reshape(1, 128)),
            "wmt0": fb(wm[:, idx[:64]].T),
            "wmt1": fb(wm[:, idx[64:]].T),
        }
        in_maps.append(m)
    return in_maps


def run(in_maps, trace=False, **kw):
    from concourse import bass_utils

    nc = _build_nc()
    return bass_utils.run_bass_kernel_spmd(
        nc, in_maps, core_ids=list(range(8)), trace=trace, **kw
    )


def gather(results, bm):
    bm = np.asarray(bm, dtype=np.float32)
    outs = [np.asarray(r["out"], dtype=np.float32) for r in results]
    return np.stack([outs[2 * b] + outs[2 * b + 1] + bm[:, None] for b in range(B)])


def kernel(query, key, value, wq, bq, wk, bk, wv, bv, wm, bm):
    in_maps = make_in_maps(query, key, value, wq, bq, wk, bk, wv, bv, wm, bm)
    res = run(in_maps)
    return gather(res.results, bm)



# revision 7
# speedup vs baseline: 1.2207x; 1.2207x over previous
"""MultiHeadedAttention Trainium2 Bass kernel (v2).

Full inputs in, full output out. 8 cores = 4 batches x 2 head-pairs.

Per-core structure (all matmuls bf16, fp32 PSUM):
  - K/Q projections -> k_sb/q_sb [128, 2048] (2 heads x 64 chans stacked).
    bk is dropped (cancels in softmax); bq applied via ACT Copy+bias.
    Scores scale 1/8 and log2(e) are folded into the K weights on host, so
    the scores PSUM holds t = s*log2(e)/8 and exp(s/8) = 2^t.
  - V^T tiles vt [128(m), mb, h, 65] via per-mb matmuls with xv as the
    stationary; col 64 is ones (softmax denominator rides along in the
    x-accumulation).  bv is dropped on device: sum_m prob = 1 makes its
    contribution wm@bv, added on host in gather().
  - Attention over 4 n-chunks of 512, 16 m-blocks of 128:
      scores: the two heads' K=64 matmuls row-packed via tile_position
        (rows 0-63 / 64-127) into one [128, 1024] PSUM pair -> 512 cycles.
      exp: one instruction per iter over the [128, 1024] pair, alternating
        between ScalarE (ACT Exp, scale=ln2) and VectorE (Schraudolph bf16
        bit-trick: int16(round(128*t + B)) reinterpreted as bf16, ~3% max
        rel err) to split the 8.4M-elem softmax across two engines.
      x-accum: per head px[65, 512] += vt^T @ pt over m-blocks; row 64 = sums.
  - Normalize: sums row -> DRAM -> [128, 8] -> DVE reciprocal (cheap shape)
    -> DRAM -> partition-broadcast DMA -> r_bc; xh = px * r_bc (bf16).
  - Out-projection: per (oc, chunk) two accumulated K=64 matmuls (h0+h1),
    PSUM -> SBUF copy (ACT/DVE alternating) -> DMA out.
Host pre-casts to bf16, sums the two per-batch partials and adds
bm + wm @ bv in fp32.
"""

import sys

if "/opt/trn_rl_repo" not in sys.path:
    sys.path.insert(0, "/opt/trn_rl_repo")

import numpy as np
import ml_dtypes

BF = ml_dtypes.bfloat16

B, D, N, H = 4, 256, 2048, 4
DIM = D // H  # 64
NW = 4  # 512-wide input windows
MB = 16  # 128-wide m blocks
NC = 4  # 512-wide n chunks

ALPHA = float(np.log2(np.e) / 8.0)  # folded into wk/host
LN2 = float(np.log(2.0))
C_SCH = 0.0430
TS_SCALE = 128.0
TS_BIAS = 128.0 * (127.0 - C_SCH) + 0.5  # +0.5: truncation -> round
# iterations (by mb index) whose exp runs on the DVE via the bit-trick
DVE_MB = (1, 3, 6, 8, 10, 13, 15)

_CACHE = {}


def _emit(ctx, tc, io):
    import concourse.bass as bass
    import concourse.mybir as mybir

    nc = tc.nc
    f32 = mybir.dt.float32
    bf16 = mybir.dt.bfloat16
    i16 = mybir.dt.int16
    EXP = mybir.ActivationFunctionType.Exp
    COPY = mybir.ActivationFunctionType.Copy
    MUL = mybir.AluOpType.mult
    ADD = mybir.AluOpType.add

    const = ctx.enter_context(tc.tile_pool(name="const", bufs=1))
    xin = ctx.enter_context(tc.tile_pool(name="xin", bufs=4))
    big = ctx.enter_context(tc.tile_pool(name="big", bufs=1))
    ptp = ctx.enter_context(tc.tile_pool(name="ptp", bufs=3))
    xhp = ctx.enter_context(tc.tile_pool(name="xhp", bufs=8))
    work = ctx.enter_context(tc.tile_pool(name="work", bufs=2))
    outp = ctx.enter_context(tc.tile_pool(name="outp", bufs=3))
    psA = ctx.enter_context(tc.tile_pool(name="psA", bufs=2, space="PSUM"))
    psX = ctx.enter_context(tc.tile_pool(name="psX", bufs=4, space="PSUM"))
    dpool = ctx.enter_context(tc.tile_pool(name="dpool", bufs=2, space="DRAM"))

    # ---- constants / weights ----
    wqt_sb = const.tile([128, 2, 128], bf16, tag="wqt")
    nc.sync.dma_start(wqt_sb, io["wqt"].rearrange("(c p) o -> p c o", p=128))
    wkt_sb = const.tile([128, 2, 128], bf16, tag="wkt")
    nc.sync.dma_start(wkt_sb, io["wkt"].rearrange("(c p) o -> p c o", p=128))
    wvt_sb = const.tile([128, 2, 128], bf16, tag="wvt")
    nc.sync.dma_start(wvt_sb, io["wvt"].rearrange("(c p) o -> p c o", p=128))
    wmt0_sb = const.tile([64, 256], bf16, tag="wmt0")
    nc.sync.dma_start(wmt0_sb, io["wmt0"])
    wmt1_sb = const.tile([64, 256], bf16, tag="wmt1")
    nc.sync.dma_start(wmt1_sb, io["wmt1"])
    bq_sb = const.tile([128, 1], f32, tag="bq")
    nc.sync.dma_start(bq_sb, io["bq"])

    wu_a = const.tile([128, 128], bf16, tag="wu_a")
    nc.gpsimd.memset(wu_a, 0.0)
    wu_b = const.tile([128, 512], bf16, tag="wu_b")
    nc.gpsimd.memset(wu_b, 0.0)
    junk = const.tile([128, 2], f32, tag="junk")

    # ACT table load for Exp happens during the input-DMA ramp
    nc.scalar.activation(junk[:, 0:1], wu_a[:, 0:1], EXP)

    # PE warmup: trip the HAM clock gate (1.2 -> 2.4 GHz) during input DMA
    wu_ps = psA.tile([128, 1024], f32, tag="ps", name="wu_ps")
    for _ in range(12):
        nc.tensor.matmul(wu_ps[:, 0:512], lhsT=wu_a, rhs=wu_b, start=True, stop=True)

    # ---- input loads, 3 HWDGE rings ----
    xq_t, xk_t, xv_t = [], [], []
    eng = {"xq": nc.sync, "xk": nc.scalar, "xv": nc.gpsimd}
    for w in range(NW):
        for name, lst in (("xk", xk_t), ("xq", xq_t), ("xv", xv_t)):
            t = xin.tile([128, 2, 512], bf16, tag=name, name=f"{name}{w}")
            src = io[name].rearrange("(c p) n -> p c n", p=128)
            for hh in range(2):
                s = slice(w * 512 + hh * 256, w * 512 + (hh + 1) * 256)
                eng[name].dma_start(t[:, :, hh * 256 : (hh + 1) * 256], src[:, :, s])
            lst.append(t)

    # ---- projections ----
    q_sb = big.tile([128, 2048], bf16, tag="q")
    k_sb = big.tile([128, 2048], bf16, tag="k")

    def proj_step(xt, wt, dst, w, bias):
        ps = psA.tile([128, 1024], f32, tag="ps", name=f"psproj{w}")
        nc.tensor.matmul(ps[:, 0:512], lhsT=wt[:, 0, :], rhs=xt[w][:, 0, :], start=True, stop=False)
        nc.tensor.matmul(ps[:, 0:512], lhsT=wt[:, 1, :], rhs=xt[w][:, 1, :], start=False, stop=True)
        if bias is None:
            nc.scalar.copy(dst[:, w * 512 : (w + 1) * 512], ps[:, 0:512])
        else:
            nc.vector.tensor_scalar_add(dst[:, w * 512 : (w + 1) * 512], ps[:, 0:512], bias)

    # ---- V^T tiles ----
    vt = big.tile([128, MB, 2, 65], bf16, tag="vt")
    nc.gpsimd.memset(vt[:, :, :, 64:65], 1.0)

    def vt_step(mb):
        w, off = divmod(mb, 4)
        ms = slice(off * 128, (off + 1) * 128)
        ps = psA.tile([128, 1024], f32, tag="ps", name=f"psvt{mb}")
        pvt = ps[:, 0:128]
        nc.tensor.matmul(pvt, lhsT=xv_t[w][:, 0, ms], rhs=wvt_sb[:, 0, :], start=True, stop=False)
        nc.tensor.matmul(pvt, lhsT=xv_t[w][:, 1, ms], rhs=wvt_sb[:, 1, :], start=False, stop=True)
        nc.vector.tensor_copy(vt[:, mb, :, 0:64], pvt.rearrange("m (h d) -> m h d", h=2))

    # k first (attention needs all of k early), q w0 early so chunk 0 starts
    proj_step(xk_t, wkt_sb, k_sb, 0, None)
    proj_step(xq_t, wqt_sb, q_sb, 0, bq_sb)
    for mb in range(4):
        vt_step(mb)
    proj_step(xk_t, wkt_sb, k_sb, 1, None)
    proj_step(xk_t, wkt_sb, k_sb, 2, None)
    for mb in range(4, 8):
        vt_step(mb)
    proj_step(xk_t, wkt_sb, k_sb, 3, None)
    proj_step(xq_t, wqt_sb, q_sb, 1, bq_sb)
    for mb in range(8, MB):
        vt_step(mb)
    proj_step(xq_t, wqt_sb, q_sb, 2, bq_sb)
    proj_step(xq_t, wqt_sb, q_sb, 3, bq_sb)

    # ---- attention ----
    xh_t = {}  # (c, h) -> [64, 512] bf16 normalized x
    for c in range(NC):
        ns = slice(c * 512, (c + 1) * 512)
        px = [psX.tile([65, 512], f32, tag="px", name=f"px{c}_{h}") for h in range(2)]
        for mb in range(MB):
            msl = slice(mb * 128, (mb + 1) * 128)
            sc = psA.tile([128, 1024], f32, tag="ps", name=f"sc{c}_{mb}")
            for h in range(2):
                nc.tensor.matmul(
                    sc[:, h * 512 : (h + 1) * 512],
                    lhsT=k_sb[h * 64 : (h + 1) * 64, msl],
                    rhs=q_sb[h * 64 : (h + 1) * 64, ns],
                    start=True,
                    stop=True,
                    tile_position=(64 * h, 0),
                )
            pt = ptp.tile([128, 1024], bf16, tag="pt", name="pt")
            if mb in DVE_MB:
                nc.vector.tensor_scalar(
                    pt[:, :].bitcast(i16), sc, TS_SCALE, TS_BIAS, MUL, ADD
                )
            else:
                nc.scalar.activation(pt, sc, EXP, scale=LN2)
            for h in range(2):
                nc.tensor.matmul(
                    px[h],
                    lhsT=vt[:, mb, h, :],
                    rhs=pt[:, h * 512 : (h + 1) * 512],
                    start=(mb == 0),
                    stop=(mb == MB - 1),
                    skip_group_check=True,
                )

        # normalize chunk: sums -> 1/sums (reshaped [128,8] for a cheap
        # reciprocal) -> partition-broadcast -> multiply
        s_pair = work.tile([1, 1024], f32, tag="s_pair", name=f"s_pair{c}")
        for h in range(2):
            nc.scalar.copy(s_pair[:, h * 512 : (h + 1) * 512], px[h][64:65, :])
        s_dram = dpool.tile([1, 1024], f32, tag="s_dram", name=f"s_dram{c}")
        nc.sync.dma_start(s_dram, s_pair)
        s128 = work.tile([128, 8], f32, tag="s128", name=f"s128_{c}")
        nc.sync.dma_start(s128, s_dram.rearrange("1 (p f) -> p f", p=128))
        r128 = work.tile([128, 8], f32, tag="r128", name=f"r128_{c}")
        nc.vector.reciprocal(r128, s128)
        r_dram = dpool.tile([1, 1024], f32, tag="r_dram", name=f"r_dram{c}")
        nc.sync.dma_start(r_dram.rearrange("1 (p f) -> p f", p=128), r128)
        r_bc = work.tile([64, 2, 512], f32, tag="r_bc", name=f"r_bc{c}")
        for h in range(2):
            r_src = bass.AP(
                tensor=r_dram.tensor,
                offset=r_dram.offset + h * 512,
                ap=[[0, 64], [1, 512]],
            )
            nc.sync.dma_start(r_bc[:, h, :], r_src)
        for h in range(2):
            xh = xhp.tile([64, 512], bf16, tag="xh", name=f"xh{c}_{h}")
            nc.vector.tensor_mul(xh, px[h][0:64, :], r_bc[:, h, :])
            xh_t[(c, h)] = xh

    # ---- out projection ----
    cp_eng = [lambda o, i: nc.scalar.copy(o, i), lambda o, i: nc.vector.tensor_copy(o, i)]
    for oc in range(2):
        ocs = slice(oc * 128, (oc + 1) * 128)
        for c in range(NC):
            po = psA.tile([128, 1024], f32, tag="ps", name=f"po{oc}_{c}")[:, 0:512]
            nc.tensor.matmul(po, lhsT=wmt0_sb[:, ocs], rhs=xh_t[(c, 0)], start=True, stop=False)
            nc.tensor.matmul(po, lhsT=wmt1_sb[:, ocs], rhs=xh_t[(c, 1)], start=False, stop=True)
            ot = outp.tile([128, 512], f32, tag="ot", name="ot")
            cp_eng[(oc * NC + c) % 2](ot, po)
            nc.sync.dma_start(io["out"][ocs, c * 512 : (c + 1) * 512], ot)


def _build_nc():
    key = "nc"
    if key in _CACHE:
        return _CACHE[key]
    from contextlib import ExitStack

    import concourse.mybir as mybir
    import concourse.tile as tile
    from concourse import bacc

    f32 = mybir.dt.float32
    bf16 = mybir.dt.bfloat16
    nc = bacc.Bacc("TRN2", target_bir_lowering=False, debug=False, num_devices=8)
    io = {}
    for name, shape, dt_ in (
        ("xq", [256, 2048], bf16),
        ("xk", [256, 2048], bf16),
        ("xv", [256, 2048], bf16),
        ("wqt", [256, 128], bf16),
        ("wkt", [256, 128], bf16),
        ("wvt", [256, 128], bf16),
        ("bq", [128, 1], f32),
        ("wmt0", [64, 256], bf16),
        ("wmt1", [64, 256], bf16),
    ):
        io[name] = nc.dram_tensor(name, shape, dt_, kind="ExternalInput").ap()
    io["out"] = nc.dram_tensor("out", [256, 2048], f32, kind="ExternalOutput").ap()

    with tile.TileContext(nc) as tc:
        with ExitStack() as ctx:
            _emit(ctx, tc, io)
    nc.compile()
    _CACHE[key] = nc
    return nc


def make_in_maps(query, key, value, wq, bq, wk, bk, wv, bv, wm, bm):
    fb = lambda a: np.ascontiguousarray(np.asarray(a, dtype=np.float32)).astype(BF)
    f = lambda a: np.ascontiguousarray(np.asarray(a), dtype=np.float32)
    query, key, value = f(query), f(key), f(value)
    wq, wk, wv, wm = f(wq), f(wk), f(wv), f(wm)
    bq = f(bq)
    in_maps = []
    for c in range(8):
        b, pair = divmod(c, 2)
        hs = (2 * pair, 2 * pair + 1)
        idx = np.array([d * H + h for h in hs for d in range(DIM)])
        m = {
            "xq": fb(query[b]),
            "xk": fb(key[b]),
            "xv": fb(value[b]),
            "wqt": fb(wq[idx].T),
            "wkt": fb(wk[idx].T * ALPHA),
            "wvt": fb(wv[idx].T),
            "bq": f(bq[idx].reshape(128, 1)),
            "wmt0": fb(wm[:, idx[:64]].T),
            "wmt1": fb(wm[:, idx[64:]].T),
        }
        in_maps.append(m)
    return in_maps


def run(in_maps, trace=False, **kw):
    from concourse import bass_utils

    nc = _build_nc()
    return bass_utils.run_bass_kernel_spmd(
        nc, in_maps, core_ids=list(range(8)), trace=trace, **kw
    )


def gather(results, wm, bv, bm):
    wm = np.asarray(wm, dtype=np.float32)
    bv = np.asarray(bv, dtype=np.float32)
    bm = np.asarray(bm, dtype=np.float32)
    corr = bm + wm @ bv
    outs = [np.asarray(r["out"], dtype=np.float32) for r in results]
    return np.stack([outs[2 * b] + outs[2 * b + 1] + corr[:, None] for b in range(B)])


def kernel(query, key, value, wq, bq, wk, bk, wv, bv, wm, bm):
    in_maps = make_in_maps(query, key, value, wq, bq, wk, bk, wv, bv, wm, bm)
    res = run(in_maps)
    return gather(res.results, wm, bv, bm)


# revision 8
# speedup vs baseline: 1.4065x; 1.1522x over previous
"""MultiHeadedAttention Trainium2 Bass kernel (v3).

Full inputs in, full output out. 8 cores = 4 batches x 2 head-pairs.

Per-core structure (all matmuls bf16, fp32 PSUM):
  - K/Q projections into PER-WINDOW tiles k_w[4]/q_w[4] [128, 512] so the
    attention m-loop starts as soon as window 0 lands (fine-grained deps).
    bk is dropped (cancels in softmax); bq applied via DVE add.
    Scores scale 1/8 and log2(e) are folded into the K weights on host, so
    scores PSUM holds t = s*log2(e)/8 and exp(s/8) = 2^t.
  - V^T tiles vt_w[4] [128, 4, 2, 65] via per-mb matmuls with xv as the
    stationary; col 64 is ones (softmax denominator rides along in the
    x-accumulation).  bv is dropped on device (host adds wm@bv).
  - Attention over 4 n-chunks of 512, 16 m-blocks of 128:
      scores: two heads' K=64 matmuls row-packed via tile_position into one
        [128, 1024] PSUM pair (3-deep ring) -> 512 cycles/iter.
      exp: one instruction per iter over the pair, strictly alternating
        ScalarE (ACT Exp, scale=ln2) and VectorE (Schraudolph bf16 bit
        trick: int16(round(128*t + B)) bitcast bf16, ~3% max rel err).
      x-accum: per head px[65, 512] += vt^T @ pt over m-blocks; row 64 = sums.
    px is single-buffered (2 banks): right after the last x-accum ScalarE
    evacuates px -> SBUF so the bank frees for the next chunk.
  - Normalize: sums -> DRAM -> [128, 8] -> DVE reciprocal -> DRAM ->
    partition-broadcast DMA -> r_bc; xh = pxe * r_bc (bf16), off critical
    path.  Out-projection per chunk (2 accumulated K=64 matmuls per oc)
    emitted two chunks later so it fills the boundary lull; PSUM from the
    scores ring.  PSUM -> SBUF copy (ACT/DVE alternating) -> DMA out.
Host pre-casts to bf16, sums the two per-batch partials and adds
bm + wm @ bv in fp32.
"""

import sys

if "/opt/trn_rl_repo" not in sys.path:
    sys.path.insert(0, "/opt/trn_rl_repo")

import numpy as np
import ml_dtypes

BF = ml_dtypes.bfloat16

B, D, N, H = 4, 256, 2048, 4
DIM = D // H  # 64
NW = 4  # 512-wide input windows
MB = 16  # 128-wide m blocks
NC = 4  # 512-wide n chunks

ALPHA = float(np.log2(np.e) / 8.0)  # folded into wk on host
LN2 = float(np.log(2.0))
C_SCH = 0.0430
TS_SCALE = 128.0
TS_BIAS = 128.0 * (127.0 - C_SCH) + 0.5  # +0.5: truncation -> round
DVE_MB = (1, 3, 5, 7, 9, 11, 13, 15)  # exp iters on the DVE (alternating)

_CACHE = {}


def _emit(ctx, tc, io):
    import concourse.bass as bass
    import concourse.mybir as mybir

    nc = tc.nc
    f32 = mybir.dt.float32
    bf16 = mybir.dt.bfloat16
    i16 = mybir.dt.int16
    EXP = mybir.ActivationFunctionType.Exp
    MUL = mybir.AluOpType.mult
    ADD = mybir.AluOpType.add

    const = ctx.enter_context(tc.tile_pool(name="const", bufs=1))
    xin = ctx.enter_context(tc.tile_pool(name="xin", bufs=4))
    kqp = ctx.enter_context(tc.tile_pool(name="kqp", bufs=4))
    vtp = ctx.enter_context(tc.tile_pool(name="vtp", bufs=4))
    ptp = ctx.enter_context(tc.tile_pool(name="ptp", bufs=3))
    pxe_p = ctx.enter_context(tc.tile_pool(name="pxe", bufs=4))
    xhp = ctx.enter_context(tc.tile_pool(name="xhp", bufs=8))
    work = ctx.enter_context(tc.tile_pool(name="work", bufs=2))
    outp = ctx.enter_context(tc.tile_pool(name="outp", bufs=3))
    psA = ctx.enter_context(tc.tile_pool(name="psA", bufs=3, space="PSUM"))
    psX = ctx.enter_context(tc.tile_pool(name="psX", bufs=2, space="PSUM"))
    dpool = ctx.enter_context(tc.tile_pool(name="dpool", bufs=2, space="DRAM"))

    # ---- constants (small, land early) ----
    wqt_sb = const.tile([128, 2, 128], bf16, tag="wqt")
    nc.sync.dma_start(wqt_sb, io["wqt"].rearrange("(c p) o -> p c o", p=128))
    wkt_sb = const.tile([128, 2, 128], bf16, tag="wkt")
    nc.scalar.dma_start(wkt_sb, io["wkt"].rearrange("(c p) o -> p c o", p=128))
    wvt_sb = const.tile([128, 2, 128], bf16, tag="wvt")
    nc.gpsimd.dma_start(wvt_sb, io["wvt"].rearrange("(c p) o -> p c o", p=128))
    wmt0_sb = const.tile([64, 256], bf16, tag="wmt0")
    nc.sync.dma_start(wmt0_sb, io["wmt0"])
    wmt1_sb = const.tile([64, 256], bf16, tag="wmt1")
    nc.sync.dma_start(wmt1_sb, io["wmt1"])
    bq_sb = const.tile([128, 1], f32, tag="bq")
    nc.sync.dma_start(bq_sb, io["bq"])

    wu_a = const.tile([128, 128], bf16, tag="wu_a")
    nc.gpsimd.memset(wu_a, 0.0)
    wu_b = const.tile([128, 512], bf16, tag="wu_b")
    nc.gpsimd.memset(wu_b, 0.0)
    junk = const.tile([128, 2], f32, tag="junk")
    nc.scalar.activation(junk[:, 0:1], wu_a[:, 0:1], EXP)  # ACT table load

    # PE warmup across the input-DMA ramp (HAM clock gate release)
    wu_ps = psA.tile([128, 1024], f32, tag="ps", name="wu_ps")
    for _ in range(14):
        nc.tensor.matmul(wu_ps[:, 0:512], lhsT=wu_a, rhs=wu_b, start=True, stop=True)

    # ---- input loads, 3 HWDGE rings, window-0 chunks first ----
    xq_t, xk_t, xv_t = [], [], []
    eng = {"xq": nc.sync, "xk": nc.scalar, "xv": nc.gpsimd}
    for w in range(NW):
        for name, lst in (("xk", xk_t), ("xq", xq_t), ("xv", xv_t)):
            t = xin.tile([128, 2, 512], bf16, tag=name, name=f"{name}{w}")
            src = io[name].rearrange("(c p) n -> p c n", p=128)
            for hh in range(2):
                s = slice(w * 512 + hh * 256, w * 512 + (hh + 1) * 256)
                eng[name].dma_start(t[:, :, hh * 256 : (hh + 1) * 256], src[:, :, s])
            lst.append(t)

    # ---- projections into per-window tiles ----
    k_w, q_w = [], []

    def proj_step(xt, wt, w, bias, lst, nm):
        ps = psA.tile([128, 1024], f32, tag="ps", name=f"ps{nm}{w}")
        nc.tensor.matmul(ps[:, 0:512], lhsT=wt[:, 0, :], rhs=xt[w][:, 0, :], start=True, stop=False)
        nc.tensor.matmul(ps[:, 0:512], lhsT=wt[:, 1, :], rhs=xt[w][:, 1, :], start=False, stop=True)
        dst = kqp.tile([128, 512], bf16, tag=nm, name=f"{nm}{w}")
        if bias is None:
            nc.scalar.copy(dst, ps[:, 0:512])
        else:
            nc.vector.tensor_scalar_add(dst, ps[:, 0:512], bias)
        lst.append(dst)

    # ---- V^T tiles, per-window ----
    vt_w = []

    def vt_block(w):
        vt = vtp.tile([128, 4, 2, 65], bf16, tag="vt", name=f"vt{w}")
        nc.gpsimd.memset(vt[:, :, :, 64:65], 1.0)
        for off in range(4):
            ms = slice(off * 128, (off + 1) * 128)
            ps = psA.tile([128, 1024], f32, tag="ps", name=f"psvt{w}_{off}")
            pvt = ps[:, 0:128]
            nc.tensor.matmul(pvt, lhsT=xv_t[w][:, 0, ms], rhs=wvt_sb[:, 0, :], start=True, stop=False)
            nc.tensor.matmul(pvt, lhsT=xv_t[w][:, 1, ms], rhs=wvt_sb[:, 1, :], start=False, stop=True)
            nc.vector.tensor_copy(vt[:, off, :, 0:64], pvt.rearrange("m (h d) -> m h d", h=2))
        vt_w.append(vt)

    proj_step(xk_t, wkt_sb, 0, None, k_w, "k")
    proj_step(xq_t, wqt_sb, 0, bq_sb, q_w, "q")
    vt_block(0)
    proj_step(xk_t, wkt_sb, 1, None, k_w, "k")
    vt_block(1)
    proj_step(xk_t, wkt_sb, 2, None, k_w, "k")
    vt_block(2)
    proj_step(xk_t, wkt_sb, 3, None, k_w, "k")
    vt_block(3)
    proj_step(xq_t, wqt_sb, 1, bq_sb, q_w, "q")
    proj_step(xq_t, wqt_sb, 2, bq_sb, q_w, "q")
    proj_step(xq_t, wqt_sb, 3, bq_sb, q_w, "q")

    # ---- attention ----
    xh_t = {}
    out_done = []

    def out_proj(c):
        for oc in range(2):
            ocs = slice(oc * 128, (oc + 1) * 128)
            po = psA.tile([128, 1024], f32, tag="ps", name=f"po{oc}_{c}")[:, 0:512]
            nc.tensor.matmul(po, lhsT=wmt0_sb[:, ocs], rhs=xh_t[(c, 0)], start=True, stop=False)
            nc.tensor.matmul(po, lhsT=wmt1_sb[:, ocs], rhs=xh_t[(c, 1)], start=False, stop=True)
            ot = outp.tile([128, 512], f32, tag="ot", name="ot")
            if oc == 0:
                nc.scalar.copy(ot, po)
            else:
                nc.vector.tensor_copy(ot, po)
            nc.sync.dma_start(io["out"][ocs, c * 512 : (c + 1) * 512], ot)
        out_done.append(c)

    for c in range(NC):
        px = [psX.tile([65, 512], f32, tag="px", name=f"px{c}_{h}") for h in range(2)]
        for mb in range(MB):
            w, off = divmod(mb, 4)
            msl = slice(off * 128, (off + 1) * 128)
            sc = psA.tile([128, 1024], f32, tag="ps", name=f"sc{c}_{mb}")
            for h in range(2):
                nc.tensor.matmul(
                    sc[:, h * 512 : (h + 1) * 512],
                    lhsT=k_w[w][h * 64 : (h + 1) * 64, msl],
                    rhs=q_w[c][h * 64 : (h + 1) * 64, :],
                    start=True,
                    stop=True,
                    tile_position=(64 * h, 0),
                )
            pt = ptp.tile([128, 1024], bf16, tag="pt", name="pt")
            if mb in DVE_MB:
                nc.vector.tensor_scalar(
                    pt[:, :].bitcast(i16), sc, TS_SCALE, TS_BIAS, MUL, ADD
                )
            else:
                nc.scalar.activation(pt, sc, EXP, scale=LN2)
            for h in range(2):
                nc.tensor.matmul(
                    px[h],
                    lhsT=vt_w[w][:, off, h, :],
                    rhs=pt[:, h * 512 : (h + 1) * 512],
                    start=(mb == 0),
                    stop=(mb == MB - 1),
                    skip_group_check=True,
                )

        # evacuate px fast (frees the PSUM bank for the next chunk)
        pxe = []
        for h in range(2):
            e = pxe_p.tile([65, 512], f32, tag="pxe", name=f"pxe{c}_{h}")
            nc.scalar.copy(e, px[h])
            pxe.append(e)

        # 1/sums via DRAM-reshaped [128, 8] reciprocal, then broadcast
        s_dram = dpool.tile([1, 1024], f32, tag="s_dram", name=f"s_dram{c}")
        for h in range(2):
            nc.sync.dma_start(
                s_dram[:, h * 512 : (h + 1) * 512], pxe[h][64:65, :]
            )
        s128 = work.tile([128, 8], f32, tag="s128", name=f"s128_{c}")
        nc.sync.dma_start(s128, s_dram.rearrange("1 (p f) -> p f", p=128))
        r128 = work.tile([128, 8], f32, tag="r128", name=f"r128_{c}")
        nc.vector.reciprocal(r128, s128)
        r_dram = dpool.tile([1, 1024], f32, tag="r_dram", name=f"r_dram{c}")
        nc.sync.dma_start(r_dram.rearrange("1 (p f) -> p f", p=128), r128)
        r_bc = work.tile([64, 2, 512], f32, tag="r_bc", name=f"r_bc{c}")
        for h in range(2):
            r_src = bass.AP(
                tensor=r_dram.tensor,
                offset=r_dram.offset + h * 512,
                ap=[[0, 64], [1, 512]],
            )
            nc.sync.dma_start(r_bc[:, h, :], r_src)
        for h in range(2):
            xh = xhp.tile([64, 512], bf16, tag="xh", name=f"xh{c}_{h}")
            nc.vector.tensor_mul(xh, pxe[h][0:64, :], r_bc[:, h, :])
            xh_t[(c, h)] = xh

        # out-projection of chunk c-1 lands in this boundary lull
        if c >= 1:
            out_proj(c - 1)
    out_proj(NC - 1)


def _build_nc():
    key = "nc"
    if key in _CACHE:
        return _CACHE[key]
    from contextlib import ExitStack

    import concourse.mybir as mybir
    import concourse.tile as tile
    from concourse import bacc

    f32 = mybir.dt.float32
    bf16 = mybir.dt.bfloat16
    nc = bacc.Bacc("TRN2", target_bir_lowering=False, debug=False, num_devices=8)
    io = {}
    for name, shape, dt_ in (
        ("xq", [256, 2048], bf16),
        ("xk", [256, 2048], bf16),
        ("xv", [256, 2048], bf16),
        ("wqt", [256, 128], bf16),
        ("wkt", [256, 128], bf16),
        ("wvt", [256, 128], bf16),
        ("bq", [128, 1], f32),
        ("wmt0", [64, 256], bf16),
        ("wmt1", [64, 256], bf16),
    ):
        io[name] = nc.dram_tensor(name, shape, dt_, kind="ExternalInput").ap()
    io["out"] = nc.dram_tensor("out", [256, 2048], f32, kind="ExternalOutput").ap()

    with tile.TileContext(nc) as tc:
        with ExitStack() as ctx:
            _emit(ctx, tc, io)
    nc.compile()
    _CACHE[key] = nc
    return nc


def make_in_maps(query, key, value, wq, bq, wk, bk, wv, bv, wm, bm):
    fb = lambda a: np.ascontiguousarray(np.asarray(a, dtype=np.float32)).astype(BF)
    f = lambda a: np.ascontiguousarray(np.asarray(a), dtype=np.float32)
    query, key, value = f(query), f(key), f(value)
    wq, wk, wv, wm = f(wq), f(wk), f(wv), f(wm)
    bq = f(bq)
    in_maps = []
    for c in range(8):
        b, pair = divmod(c, 2)
        hs = (2 * pair, 2 * pair + 1)
        idx = np.array([d * H + h for h in hs for d in range(DIM)])
        m = {
            "xq": fb(query[b]),
            "xk": fb(key[b]),
            "xv": fb(value[b]),
            "wqt": fb(wq[idx].T),
            "wkt": fb(wk[idx].T * ALPHA),
            "wvt": fb(wv[idx].T),
            "bq": f(bq[idx].reshape(128, 1)),
            "wmt0": fb(wm[:, idx[:64]].T),
            "wmt1": fb(wm[:, idx[64:]].T),
        }
        in_maps.append(m)
    return in_maps


def run(in_maps, trace=False, **kw):
    from concourse import bass_utils

    nc = _build_nc()
    return bass_utils.run_bass_kernel_spmd(
        nc, in_maps, core_ids=list(range(8)), trace=trace, **kw
    )


def gather(results, wm, bv, bm):
    wm = np.asarray(wm, dtype=np.float32)
    bv = np.asarray(bv, dtype=np.float32)
    bm = np.asarray(bm, dtype=np.float32)
    corr = bm + wm @ bv
    outs = [np.asarray(r["out"], dtype=np.float32) for r in results]
    return np.stack([outs[2 * b] + outs[2 * b + 1] + corr[:, None] for b in range(B)])


def kernel(query, key, value, wq, bq, wk, bk, wv, bv, wm, bm):
    in_maps = make_in_maps(query, key, value, wq, bq, wk, bk, wv, bv, wm, bm)
    res = run(in_maps)
    return gather(res.results, wm, bv, bm)


# revision 10
# speedup vs baseline: 1.4807x; 1.0527x over previous
"""MultiHeadedAttention Trainium2 Bass kernel (v4).

Full inputs in, full output out. 8 cores = 4 batches x 2 head-pairs.

Per-core structure (all matmuls bf16, fp32 PSUM):
  - K/Q projections into per-window tiles k_w[4]/q_w[4] [128, 512]; their
    emission is interleaved into chunk 0's m-loop so attention streams
    behind the input DMA.  bk dropped (cancels in softmax); bq via DVE add.
    Scores scale 1/8 and log2(e) folded into K weights on host: scores
    PSUM holds t = s*log2(e)/8, exp(s/8) = 2^t.
  - V^T tiles vt_w[4] [128, 4, 2, 65]; col 64 = ones so the softmax
    denominator rides along the x-accumulation.  bv applied on host.
  - Attention: 64 global iterations g = (chunk, mb), chunk = 512 n-cols,
    mb = 128 m-rows.  The TensorE runs matmuls strictly in program order,
    so emission is software-pipelined: scores (head-pair row-packed via
    tile_position into one [128, 1024] PSUM pair, 3-deep ring) and exp are
    emitted 2 iterations ahead of the x-accums.  exp alternates ScalarE
    (ACT Exp, scale=ln2) and VectorE (Schraudolph bf16 bit trick:
    int16(round(128*t + B)) bitcast bf16, ~3% max rel err) so both engines
    split the 8.4M-elem softmax.
  - Chunk tails: px [65, 512] PSUM evacuated immediately (ACT h0 / DVE h1)
    to free the single-buffered px banks; sums -> [128, 8] SBUF->SBUF DMA
    -> DVE reciprocal -> DRAM -> partition-broadcast DMA -> r_bc;
    xh = pxe * r_bc on GPSIMD (otherwise idle).  Out-projection per chunk
    (2 accumulated K=64 matmuls per oc) lands in the next chunk boundary;
    PSUM from the scores ring; PSUM->SBUF copy ACT/DVE, DMA out.
Host sums the two per-batch partials and adds bm + wm @ bv in fp32.
"""

import sys

if "/opt/trn_rl_repo" not in sys.path:
    sys.path.insert(0, "/opt/trn_rl_repo")

import numpy as np
import ml_dtypes

BF = ml_dtypes.bfloat16

B, D, N, H = 4, 256, 2048, 4
DIM = D // H  # 64
NW = 4  # 512-wide input windows
MB = 16  # 128-wide m blocks
NC = 4  # 512-wide n chunks
G = NC * MB  # 64 global iterations
LA = 2  # scores/exp emitted this many iterations ahead of x-accums

ALPHA = float(np.log2(np.e) / 8.0)  # folded into wk on host
LN2 = float(np.log(2.0))
C_SCH = 0.0430
TS_SCALE = 128.0
TS_BIAS = 128.0 * (127.0 - C_SCH) + 0.5  # +0.5: truncation -> round

_CACHE = {}


def _emit(ctx, tc, io):
    import concourse.bass as bass
    import concourse.mybir as mybir

    nc = tc.nc
    f32 = mybir.dt.float32
    bf16 = mybir.dt.bfloat16
    i16 = mybir.dt.int16
    EXP = mybir.ActivationFunctionType.Exp
    MUL = mybir.AluOpType.mult
    ADD = mybir.AluOpType.add

    const = ctx.enter_context(tc.tile_pool(name="const", bufs=1))
    xin = ctx.enter_context(tc.tile_pool(name="xin", bufs=4))
    kqp = ctx.enter_context(tc.tile_pool(name="kqp", bufs=4))
    vtp = ctx.enter_context(tc.tile_pool(name="vtp", bufs=4))
    ptp = ctx.enter_context(tc.tile_pool(name="ptp", bufs=3))
    pxe_p = ctx.enter_context(tc.tile_pool(name="pxe", bufs=4))
    xhp = ctx.enter_context(tc.tile_pool(name="xhp", bufs=8))
    work = ctx.enter_context(tc.tile_pool(name="work", bufs=2))
    outp = ctx.enter_context(tc.tile_pool(name="outp", bufs=3))
    psA = ctx.enter_context(tc.tile_pool(name="psA", bufs=3, space="PSUM"))
    psX = ctx.enter_context(tc.tile_pool(name="psX", bufs=2, space="PSUM"))
    dpool = ctx.enter_context(tc.tile_pool(name="dpool", bufs=2, space="DRAM"))

    # ---- constants (small, land early) ----
    wqt_sb = const.tile([128, 2, 128], bf16, tag="wqt")
    nc.sync.dma_start(wqt_sb, io["wqt"].rearrange("(c p) o -> p c o", p=128))
    wkt_sb = const.tile([128, 2, 128], bf16, tag="wkt")
    nc.scalar.dma_start(wkt_sb, io["wkt"].rearrange("(c p) o -> p c o", p=128))
    wvt_sb = const.tile([128, 2, 128], bf16, tag="wvt")
    nc.gpsimd.dma_start(wvt_sb, io["wvt"].rearrange("(c p) o -> p c o", p=128))
    wmt0_sb = const.tile([64, 256], bf16, tag="wmt0")
    nc.sync.dma_start(wmt0_sb, io["wmt0"])
    wmt1_sb = const.tile([64, 256], bf16, tag="wmt1")
    nc.sync.dma_start(wmt1_sb, io["wmt1"])
    bq_sb = const.tile([128, 1], f32, tag="bq")
    nc.sync.dma_start(bq_sb, io["bq"])

    wu_a = const.tile([128, 128], bf16, tag="wu_a")
    nc.gpsimd.memset(wu_a, 0.0)
    wu_b = const.tile([128, 512], bf16, tag="wu_b")
    nc.gpsimd.memset(wu_b, 0.0)
    junk = const.tile([128, 2], f32, tag="junk")
    nc.scalar.activation(junk[:, 0:1], wu_a[:, 0:1], EXP)  # ACT table load

    # PE warmup across the input-DMA ramp (HAM clock gate release)
    wu_ps = psA.tile([128, 1024], f32, tag="ps", name="wu_ps")
    for _ in range(11):
        nc.tensor.matmul(wu_ps[:, 0:512], lhsT=wu_a, rhs=wu_b, start=True, stop=True)

    # ---- input loads, 3 HWDGE rings, window order ----
    xq_t, xk_t, xv_t = [], [], []
    eng = {"xq": nc.sync, "xk": nc.scalar, "xv": nc.gpsimd}
    for w in range(NW):
        for name, lst in (("xk", xk_t), ("xq", xq_t), ("xv", xv_t)):
            t = xin.tile([128, 2, 512], bf16, tag=name, name=f"{name}{w}")
            src = io[name].rearrange("(c p) n -> p c n", p=128)
            for hh in range(2):
                s = slice(w * 512 + hh * 256, w * 512 + (hh + 1) * 256)
                eng[name].dma_start(t[:, :, hh * 256 : (hh + 1) * 256], src[:, :, s])
            lst.append(t)

    # ---- projection / V^T emitters ----
    k_w, q_w, vt_w = [], [], []

    def proj_step(xt, wt, w, bias, lst, nm):
        ps = psA.tile([128, 1024], f32, tag="ps", name=f"ps{nm}{w}")
        nc.tensor.matmul(ps[:, 0:512], lhsT=wt[:, 0, :], rhs=xt[w][:, 0, :], start=True, stop=False)
        nc.tensor.matmul(ps[:, 0:512], lhsT=wt[:, 1, :], rhs=xt[w][:, 1, :], start=False, stop=True)
        dst = kqp.tile([128, 512], bf16, tag=nm, name=f"{nm}{w}")
        if bias is None:
            nc.scalar.copy(dst, ps[:, 0:512])
        else:
            nc.vector.tensor_scalar_add(dst, ps[:, 0:512], bias)
        lst.append(dst)

    def vt_block(w):
        vt = vtp.tile([128, 4, 2, 65], bf16, tag="vt", name=f"vt{w}")
        nc.gpsimd.memset(vt[:, :, :, 64:65], 1.0)
        for off in range(4):
            ms = slice(off * 128, (off + 1) * 128)
            ps = psA.tile([128, 1024], f32, tag="ps", name=f"psvt{w}_{off}")
            pvt = ps[:, 0:128]
            nc.tensor.matmul(pvt, lhsT=xv_t[w][:, 0, ms], rhs=wvt_sb[:, 0, :], start=True, stop=False)
            nc.tensor.matmul(pvt, lhsT=xv_t[w][:, 1, ms], rhs=wvt_sb[:, 1, :], start=False, stop=True)
            nc.vector.tensor_copy(vt[:, off, :, 0:64], pvt.rearrange("m (h d) -> m h d", h=2))
        vt_w.append(vt)

    # ---- software-pipelined attention ----
    sc_t, pt_t, px_t, pxe_t, xh_t = {}, {}, {}, {}, {}
    out_done = []

    def emit_sc_exp(g):
        c, mb = divmod(g, MB)
        w, off = divmod(mb, 4)
        msl = slice(off * 128, (off + 1) * 128)
        sc = psA.tile([128, 1024], f32, tag="ps", name=f"sc{c}_{mb}")
        for h in range(2):
            nc.tensor.matmul(
                sc[:, h * 512 : (h + 1) * 512],
                lhsT=k_w[w][h * 64 : (h + 1) * 64, msl],
                rhs=q_w[c][h * 64 : (h + 1) * 64, :],
                start=True,
                stop=True,
                tile_position=(64 * h, 0),
            )
        pt = ptp.tile([128, 1024], bf16, tag="pt", name="pt")
        if mb % 2 == 1:
            nc.vector.tensor_scalar(pt[:, :].bitcast(i16), sc, TS_SCALE, TS_BIAS, MUL, ADD)
        else:
            nc.scalar.activation(pt, sc, EXP, scale=LN2)
        sc_t[g], pt_t[g] = sc, pt

    def emit_xacc(g):
        c, mb = divmod(g, MB)
        w, off = divmod(mb, 4)
        if mb == 0:
            px_t[c] = [psX.tile([65, 512], f32, tag="px", name=f"px{c}_{h}") for h in range(2)]
        pt = pt_t.pop(g)
        for h in range(2):
            nc.tensor.matmul(
                px_t[c][h],
                lhsT=vt_w[w][:, off, h, :],
                rhs=pt[:, h * 512 : (h + 1) * 512],
                start=(mb == 0),
                stop=(mb == MB - 1),
                skip_group_check=True,
            )

    def out_proj(c):
        for oc in range(2):
            ocs = slice(oc * 128, (oc + 1) * 128)
            po = psA.tile([128, 1024], f32, tag="ps", name=f"po{oc}_{c}")[:, 0:512]
            nc.tensor.matmul(po, lhsT=wmt0_sb[:, ocs], rhs=xh_t[(c, 0)], start=True, stop=False)
            nc.tensor.matmul(po, lhsT=wmt1_sb[:, ocs], rhs=xh_t[(c, 1)], start=False, stop=True)
            ot = outp.tile([128, 512], f32, tag="ot", name="ot")
            if oc == 0:
                nc.scalar.copy(ot, po)
            else:
                nc.vector.tensor_copy(ot, po)
            nc.sync.dma_start(io["out"][ocs, c * 512 : (c + 1) * 512], ot)
        out_done.append(c)

    def chunk_tail(c):
        px = px_t.pop(c)
        pxe = []
        for h in range(2):
            e = pxe_p.tile([65, 512], f32, tag="pxe", name=f"pxe{c}_{h}")
            if h == 0:
                nc.scalar.copy(e, px[h])
            else:
                nc.vector.tensor_copy(e, px[h])
            pxe.append(e)
        # 1/sums via DRAM-reshaped [128, 8] reciprocal, then broadcast
        s_dram = dpool.tile([1, 1024], f32, tag="s_dram", name=f"s_dram{c}")
        for h in range(2):
            nc.sync.dma_start(s_dram[:, h * 512 : (h + 1) * 512], pxe[h][64:65, :])
        s128 = work.tile([128, 8], f32, tag="s128", name=f"s128_{c}")
        nc.sync.dma_start(s128, s_dram.rearrange("1 (p f) -> p f", p=128))
        r128 = work.tile([128, 8], f32, tag="r128", name=f"r128_{c}")
        nc.vector.reciprocal(r128, s128)
        r_dram = dpool.tile([1, 1024], f32, tag="r_dram", name=f"r_dram{c}")
        nc.sync.dma_start(r_dram.rearrange("1 (p f) -> p f", p=128), r128)
        r_bc = work.tile([64, 2, 512], f32, tag="r_bc", name=f"r_bc{c}")
        for h in range(2):
            r_src = bass.AP(
                tensor=r_dram.tensor,
                offset=r_dram.offset + h * 512,
                ap=[[0, 64], [1, 512]],
            )
            nc.sync.dma_start(r_bc[:, h, :], r_src)
        for h in range(2):
            xh = xhp.tile([64, 512], bf16, tag="xh", name=f"xh{c}_{h}")
            nc.gpsimd.tensor_mul(xh, pxe[h][0:64, :], r_bc[:, h, :])
            xh_t[(c, h)] = xh
        pxe_t[c] = pxe

    # prelude: window 0 of everything, then 2 iterations of lookahead
    proj_step(xk_t, wkt_sb, 0, None, k_w, "k")
    proj_step(xq_t, wqt_sb, 0, bq_sb, q_w, "q")
    vt_block(0)
    for g in range(LA):
        emit_sc_exp(g)

    for g in range(G):
        ga = g + LA
        if ga < G:
            c, mb = divmod(ga, MB)
            if c == 0 and mb in (4, 8, 12):
                proj_step(xk_t, wkt_sb, mb // 4, None, k_w, "k")
                vt_block(mb // 4)
            if mb == 0 and c in (1, 2, 3):
                proj_step(xq_t, wqt_sb, c, bq_sb, q_w, "q")
            emit_sc_exp(ga)
        emit_xacc(g)
        if g % MB == MB - 1:
            c = g // MB
            chunk_tail(c)
            if c >= 1:
                out_proj(c - 1)
    out_proj(NC - 1)


def _build_nc():
    key = "nc"
    if key in _CACHE:
        return _CACHE[key]
    from contextlib import ExitStack

    import concourse.mybir as mybir
    import concourse.tile as tile
    from concourse import bacc

    f32 = mybir.dt.float32
    bf16 = mybir.dt.bfloat16
    nc = bacc.Bacc("TRN2", target_bir_lowering=False, debug=False, num_devices=8)
    io = {}
    for name, shape, dt_ in (
        ("xq", [256, 2048], bf16),
        ("xk", [256, 2048], bf16),
        ("xv", [256, 2048], bf16),
        ("wqt", [256, 128], bf16),
        ("wkt", [256, 128], bf16),
        ("wvt", [256, 128], bf16),
        ("bq", [128, 1], f32),
        ("wmt0", [64, 256], bf16),
        ("wmt1", [64, 256], bf16),
    ):
        io[name] = nc.dram_tensor(name, shape, dt_, kind="ExternalInput").ap()
    io["out"] = nc.dram_tensor("out", [256, 2048], f32, kind="ExternalOutput").ap()

    with tile.TileContext(nc) as tc:
        with ExitStack() as ctx:
            _emit(ctx, tc, io)
    nc.compile()
    _CACHE[key] = nc
    return nc


def make_in_maps(query, key, value, wq, bq, wk, bk, wv, bv, wm, bm):
    fb = lambda a: np.ascontiguousarray(np.asarray(a, dtype=np.float32)).astype(BF)
    f = lambda a: np.ascontiguousarray(np.asarray(a), dtype=np.float32)
    query, key, value = f(query), f(key), f(value)
    wq, wk, wv, wm = f(wq), f(wk), f(wv), f(wm)
    bq = f(bq)
    in_maps = []
    for c in range(8):
        b, pair = divmod(c, 2)
        hs = (2 * pair, 2 * pair + 1)
        idx = np.array([d * H + h for h in hs for d in range(DIM)])
        m = {
            "xq": fb(query[b]),
            "xk": fb(key[b]),
            "xv": fb(value[b]),
            "wqt": fb(wq[idx].T),
            "wkt": fb(wk[idx].T * ALPHA),
            "wvt": fb(wv[idx].T),
            "bq": f(bq[idx].reshape(128, 1)),
            "wmt0": fb(wm[:, idx[:64]].T),
            "wmt1": fb(wm[:, idx[64:]].T),
        }
        in_maps.append(m)
    return in_maps


def run(in_maps, trace=False, **kw):
    from concourse import bass_utils

    nc = _build_nc()
    return bass_utils.run_bass_kernel_spmd(
        nc, in_maps, core_ids=list(range(8)), trace=trace, **kw
    )


def gather(results, wm, bv, bm):
    wm = np.asarray(wm, dtype=np.float32)
    bv = np.asarray(bv, dtype=np.float32)
    bm = np.asarray(bm, dtype=np.float32)
    corr = bm + wm @ bv
    outs = [np.asarray(r["out"], dtype=np.float32) for r in results]
    return np.stack([outs[2 * b] + outs[2 * b + 1] + corr[:, None] for b in range(B)])


def kernel(query, key, value, wq, bq, wk, bk, wv, bv, wm, bm):
    in_maps = make_in_maps(query, key, value, wq, bq, wk, bk, wv, bv, wm, bm)
    res = run(in_maps)
    return gather(res.results, wm, bv, bm)
